# revision 23
# baseline (speedup 1.0000x reference)
"""DigitCaps (capsule routing) Trainium2 kernel.

Contract: kernel(x, W) takes the FULL inputs
  x: [512, 32, 8, 6, 6] fp32, W: [1, 10, 1152, 16, 8] fp32
and returns v: [512, 10, 16] fp32, running on 8 NeuronCores with the
batch sharded 8 ways (64 per core) and W replicated.

Math (per reference):
  xr = x.reshape(B, 1152, 8)
  u[b,o,i,w] = sum_s W[o,i,w,s] xr[b,i,s]
  3 routing iterations of: c = softmax_o(beta); s = sum_i c*u;
  v = squash(s); beta += sum_w u*v   (last iteration's beta update is dead)

Kernel never materializes u. Per iteration:
  s_o[b,w]   = sum_{i,s} W[o,i,w,s] * (e[b,o,i] * x[b,i,s]/Z[b,i])   (PE)
  z[b,o,i,s] = sum_w W[o,i,w,s] * v[b,o,w]                            (PE,
               block-diagonal k=64=(s%4,w) matmuls with a delta-masked
               replicated-v operand)
  a[b,o,i]   = sum_s x[b,i,s] * z[b,o,i,s]        (DVE mult + pair tree)
Iteration 1 uses c = 1/10 (softmax of zeros), folded into squash input.

Layouts (i-partitioned; p is the SBUF partition index):
  XB [p,c,s,b]      = xr[b, 128c+p, s]                  bf16
  WA [p,c,o,s,w]    = W[o, 128c+p, w, s]                bf16
  W2 [64*(s//4)+16*(s%4)+w, o, i] = W[o,i,w,s]          bf16
All heavy compute in bf16 with fp32 PSUM accumulation; routing logits
(a/beta) in fp32.
"""

import sys

import numpy as np

if "/opt/trn_rl_repo" not in sys.path:
    sys.path.insert(0, "/opt/trn_rl_repo")

import ml_dtypes

import concourse.bass as bass
import concourse.tile as tile
from concourse import bacc, mybir
from concourse.bass_utils import run_bass_kernel_spmd

BF = ml_dtypes.bfloat16
F32 = mybir.dt.float32
BF16 = mybir.dt.bfloat16

B, O, I, OW, S = 512, 10, 1152, 16, 8
NCORES = 8
BL = B // NCORES          # 64 batches per core
C9 = I // 128             # 9 i-chunks
AF = mybir.ActivationFunctionType
ALU = mybir.AluOpType


# ---------------------------------------------------------------------------
# device program
# ---------------------------------------------------------------------------

def _emit(nc, tc, t, ctx):
    """Emit the whole per-core program. `t` holds dram tensor handles."""
    P = ctx.enter_context(tc.tile_pool(name="pers", bufs=1))
    WK = ctx.enter_context(tc.tile_pool(name="work", bufs=2))
    SM = ctx.enter_context(tc.tile_pool(name="small", bufs=1))

    # ---- persistent SBUF tensors -----------------------------------------
    XB = P.tile([128, C9, S, BL], BF16, name="XB")
    WA = P.tile([128, C9, O, S, OW], BF16, name="WA")
    W2 = P.tile([128, O, I], BF16, name="W2")
    ID64 = P.tile([64, 64], F32, name="ID64")
    ID128 = P.tile([128, 16], F32, name="ID128")
    V8D = P.tile([128, O, S, BL], BF16, name="V8D")
    A1 = P.tile([128, C9, O, BL], F32, name="A1")      # beta accumulator
    E = P.tile([128, C9, O, BL], BF16, name="E")       # exp(beta)
    ZS = P.tile([128, C9, BL], F32, name="ZS")         # sum_o exp
    RZ = P.tile([128, C9, BL], F32, name="RZ")
    XP = P.tile([128, C9, S, BL], BF16, name="XP")     # x / Z

    nc.sync.dma_start(XB[:], t["xb"][:])
    nc.sync.dma_start(WA[:], t["wa"][:])
    nc.sync.dma_start(W2[:], t["w2"][:])
    nc.sync.dma_start(ID64[:], t["id64"][:])
    nc.sync.dma_start(ID128[:], t["id128"][:])
    nc.gpsimd.memset(V8D[:], 0.0)

    # ---- small helpers ----------------------------------------------------
    def squash_from_bT(s_bT, scale, it):
        """s_bT: [64, O, OW] fp32 SBUF -> v_sb [64, O, OW] fp32."""
        n2 = SM.tile([64, O], F32, name=f"n2_{it}", tag=f"n2_{it}")
        sq = SM.tile([64, O], F32, name=f"sq_{it}", tag=f"sq_{it}")
        den = SM.tile([64, O], F32, name=f"den_{it}", tag=f"den_{it}")
        rden = SM.tile([64, O], F32, name=f"rden_{it}", tag=f"rden_{it}")
        gam = SM.tile([64, O], F32, name=f"gam_{it}", tag=f"gam_{it}")
        v_sb = SM.tile([64, O, OW], F32, name=f"v_sb_{it}", tag=f"v_sb_{it}")
        if scale != 1.0:
            s2t = SM.tile([64, O, OW], F32, name=f"s2t_{it}", tag=f"s2t_{it}")
            nc.scalar.mul(s2t[:], s_bT[:], scale)
            s_bT = s2t
        ssq = SM.tile([64, O, OW], F32, name=f"ssq_{it}", tag=f"ssq_{it}")
        nc.scalar.square(ssq[:], s_bT[:])
        nc.vector.reduce_sum(
            n2[:].unsqueeze(2), ssq[:], axis=mybir.AxisListType.X)
        nc.scalar.sqrt(sq[:], n2[:])
        n2p1 = SM.tile([64, O], F32, name=f"n2p1_{it}", tag=f"n2p1_{it}")
        nc.vector.tensor_scalar_add(n2p1[:], n2[:], 1.0)
        nc.vector.tensor_mul(den[:], n2p1[:], sq[:])
        nc.vector.reciprocal_approx_fast(rden[:], den[:])
        nc.vector.tensor_mul(gam[:], n2[:], rden[:])
        nc.vector.tensor_mul(
            v_sb[:], s_bT[:],
            gam[:].unsqueeze(2).broadcast_to([64, O, OW]))
        return v_sb

    def build_v8d(v_sb, it):
        """v_sb [64, O, OW] fp32 -> V8D block-diag (bf16).

        V8D[64*(s//4)+16*(s%4)+w, o, s', b] = v[b,o,w] * (s == s')."""
        with tc.tile_pool(name=f"vps_{it}", space="PSUM", bufs=1) as vp:
            vtp = vp.tile([16, O, BL], F32, name=f"vtp_{it}")
            for o in range(O):
                # [64, 16] -> [16, 64]
                nc.tensor.transpose(vtp[:, o, :], v_sb[:, o, :], ID64[:])
            vT = SM.tile([16, O, BL], BF16, name=f"vT_{it}", tag=f"vT_{it}")
            nc.scalar.copy(vT[:], vtp[:])
        for s in range(S):
            p0 = 64 * (s // 4) + 16 * (s % 4)
            nc.sync.dma_start(V8D[p0:p0 + 16, :, s, :], vT[:])

    def agreement(it, accumulate):
        """z-matmuls + zx mult + s-tree -> a; writes/accumulates into A1."""
        with tc.tile_pool(name=f"zps_{it}", space="PSUM", bufs=2) as zp:
            for c in range(C9):
                zxw = WK.tile([128, O, S, BL], BF16, name=f"zxw{it}_{c}",
                              tag="zxw")
                xbc4 = XB[:, c, :, :].unsqueeze(1)
                for wi, (o0, on) in enumerate(((0, 4), (4, 4), (8, 2))):
                    zq = zp.tile([128, 4, S, BL], F32,
                                 name=f"zq{it}_{c}_{wi}", tag="zq")
                    for oo in range(on):
                        o = o0 + oo
                        nc.tensor.matmul(
                            zq[:, oo, :, :],
                            W2[:, o, 128 * c:128 * (c + 1)],
                            V8D[:, o, :, :],
                        )
                    if wi == 0:
                        # DVE reads PSUM directly
                        nc.vector.tensor_mul(
                            zxw[:, o0:o0 + on, :, :],
                            zq[:, 0:on, :, :],
                            xbc4.broadcast_to([128, on, S, BL]))
                    else:
                        zsb = WK.tile([128, 4, S, BL], BF16,
                                      name=f"zsb{it}_{c}_{wi}", tag="zsb")
                        nc.scalar.copy(zsb[:, 0:on, :, :], zq[:, 0:on, :, :])
                        nc.vector.tensor_mul(
                            zxw[:, o0:o0 + on, :, :],
                            zsb[:, 0:on, :, :],
                            xbc4.broadcast_to([128, on, S, BL]))
                # pair tree over s
                t4 = WK.tile([128, O, 4, BL], BF16, name=f"t4_{it}_{c}",
                             tag="t4", bufs=1)
                t2 = WK.tile([128, O, 2, BL], BF16, name=f"t2_{it}_{c}",
                             tag="t2", bufs=1)
                nc.vector.tensor_add(t4[:], zxw[:, :, 0:4, :],
                                     zxw[:, :, 4:8, :])
                nc.vector.tensor_add(t2[:], t4[:, :, 0:2, :],
                                     t4[:, :, 2:4, :])
                if not accumulate:
                    nc.vector.tensor_add(A1[:, c, :, :], t2[:, :, 0, :],
                                         t2[:, :, 1, :])
                else:
                    a2c = WK.tile([128, O, BL], F32, name=f"a2c_{c}",
                                  tag="a2c")
                    nc.vector.tensor_add(a2c[:], t2[:, :, 0, :],
                                         t2[:, :, 1, :])
                    nc.vector.tensor_add(A1[:, c, :, :], A1[:, c, :, :],
                                         a2c[:])

    def softmax_xp():
        """E = exp(A1); ZS = sum_o E; RZ = 1/ZS; XP = XB * RZ."""
        nc.scalar.activation(E[:], A1[:], AF.Exp)
        t5 = WK.tile([128, C9, 5, BL], F32, name="t5", tag="zxw")
        u2 = SM.tile([128, C9, 2, BL], F32, name="u2", tag="u2")
        u1 = SM.tile([128, C9, BL], F32, name="u1", tag="u1")
        nc.vector.tensor_add(t5[:], E[:, :, 0:5, :], E[:, :, 5:10, :])
        nc.vector.tensor_add(u2[:], t5[:, :, 0:2, :], t5[:, :, 2:4, :])
        nc.vector.tensor_add(u1[:], u2[:, :, 0, :], u2[:, :, 1, :])
        nc.vector.tensor_add(ZS[:], u1[:], t5[:, :, 4, :])
        nc.vector.reciprocal_approx_fast(
            RZ[:].rearrange("p c b -> p (c b)"),
            ZS[:].rearrange("p c b -> p (c b)"))
        nc.vector.tensor_mul(
            XP[:], XB[:],
            RZ[:].unsqueeze(2).broadcast_to([128, C9, S, BL]))

    def s_iter1():
        """s1 via rhs=XB (c = 1/10 folded into squash scale)."""
        with tc.tile_pool(name="sps1", space="PSUM", bufs=1) as sp1p:
            sp1 = sp1p.tile([64, O, OW], F32, name="sp1")
            for c in range(C9):
                for s in range(S):
                    nc.tensor.matmul(
                        sp1[:],
                        XB[:, c, s, :],
                        WA[:, c, :, s, :],
                        start=(c == 0 and s == 0),
                        stop=(c == C9 - 1 and s == S - 1),
                    )
            s_bT = SM.tile([64, O, OW], F32, name="s_bT1", tag="s_bT")
            nc.scalar.copy(s_bT[:], sp1[:])
        return s_bT

    def s_iter23(it):
        """y = XP*E per chunk; 720 matmuls in two o-passes (8 + 2) so each
        o-accumulation group owns a whole PSUM bank."""
        s_sbs = []
        for o0, on in ((0, 8), (8, 2)):
            with tc.tile_pool(name=f"sps{it}_{o0}", space="PSUM",
                              bufs=1) as spp:
                sp = spp.tile([16, on, 512], F32, name=f"sp{it}_{o0}")
                for c in range(C9):
                    y = WK.tile([128, on, S, BL], BF16,
                                name=f"y{it}_{o0}_{c}", tag=f"y{on}")
                    nc.vector.tensor_mul(
                        y[:],
                        XP[:, c, :, :].unsqueeze(1).broadcast_to(
                            [128, on, S, BL]),
                        E[:, c, o0:o0 + on, :].unsqueeze(2).broadcast_to(
                            [128, on, S, BL]))
                    for s in range(S):
                        for oo in range(on):
                            nc.tensor.matmul(
                                sp[:, oo, 0:BL],
                                WA[:, c, o0 + oo, s, :],
                                y[:, oo, s, :],
                                start=(c == 0 and s == 0),
                                stop=(c == C9 - 1 and s == S - 1),
                            )
                s_sb = SM.tile([16, on, BL], F32, name=f"s_sb{it}_{o0}",
                               tag=f"s_sb{it}_{o0}")
                nc.scalar.copy(s_sb[:], sp[:, :, 0:BL])
                s_sbs.append((o0, on, s_sb))
        with tc.tile_pool(name=f"tps{it}", space="PSUM", bufs=1) as tpp:
            tsp = tpp.tile([64, O, OW], F32, name=f"tsp{it}")
            for o0, on, s_sb in s_sbs:
                for oo in range(on):
                    nc.tensor.transpose(
                        tsp[:, o0 + oo, :],
                        s_sb[:, oo, :],
                        ID128[0:16, :],
                    )
            s_bT = SM.tile([64, O, OW], F32, name=f"s_bT{it}",
                           tag=f"s_bT{it}")
            nc.scalar.copy(s_bT[:], tsp[:])
        return s_bT

    # ---- iteration 1 ------------------------------------------------------
    s_bT = s_iter1()
    v1 = squash_from_bT(s_bT, 0.1, 1)
    build_v8d(v1, 1)
    agreement(1, accumulate=False)

    # ---- iteration 2 ------------------------------------------------------
    softmax_xp()
    s_bT = s_iter23(2)
    v2 = squash_from_bT(s_bT, 1.0, 2)
    build_v8d(v2, 2)
    agreement(2, accumulate=True)

    # ---- iteration 3 ------------------------------------------------------
    softmax_xp()
    s_bT = s_iter23(3)
    v3 = squash_from_bT(s_bT, 1.0, 3)
    nc.sync.dma_start(t["v"][:], v3[:])


def _build_nc():
    nc = bacc.Bacc("TRN2", target_bir_lowering=False)
    t = {
        "xb": nc.dram_tensor("xb", [128, C9, S, BL], BF16,
                             kind="ExternalInput"),
        "wa": nc.dram_tensor("wa", [128, C9, O, S, OW], BF16,
                             kind="ExternalInput"),
        "w2": nc.dram_tensor("w2", [128, O, I], BF16, kind="ExternalInput"),
        "id64": nc.dram_tensor("id64", [64, 64], F32, kind="ExternalInput"),
        "id128": nc.dram_tensor("id128", [128, 16], F32,
                                kind="ExternalInput"),
        "v": nc.dram_tensor("v", [BL, O, OW], F32, kind="ExternalOutput"),
    }
    from contextlib import ExitStack
    with tile.TileContext(nc) as tc, ExitStack() as ctx:
        _emit(nc, tc, t, ctx)
    nc.finalize()
    return nc


_NC_CACHE = {}


def _get_nc():
    if "nc" not in _NC_CACHE:
        _NC_CACHE["nc"] = _build_nc()
    return _NC_CACHE["nc"]


# ---------------------------------------------------------------------------
# host side
# ---------------------------------------------------------------------------

def _host_layouts(x, W):
    xr = np.ascontiguousarray(np.asarray(x, np.float32)).reshape(B, I, S)
    W0 = np.asarray(W, np.float32)[0]                     # [O, I, OW, S]

    xbs = []
    for k in range(NCORES):
        xc = xr[k * BL:(k + 1) * BL]
        tmp = xc.transpose(1, 2, 0)                        # [I, S, BL]
        xb = tmp.reshape(C9, 128, S, BL).transpose(1, 0, 2, 3)
        xbs.append(np.ascontiguousarray(xb).astype(BF))

    wa = W0.transpose(1, 0, 3, 2)                          # [I, O, S, OW]
    wa = wa.reshape(C9, 128, O, S, OW).transpose(1, 0, 2, 3, 4)
    wa = np.ascontiguousarray(wa).astype(BF)

    w2 = np.zeros((128, O, I), np.float32)
    for s in range(S):
        p0 = 64 * (s // 4) + 16 * (s % 4)
        w2[p0:p0 + OW] = W0[:, :, :, s].transpose(2, 0, 1)
    w2 = w2.astype(BF)

    id64 = np.eye(64, dtype=np.float32)
    id128 = np.zeros((128, 16), np.float32)
    for g in range(4):
        id128[32 * g:32 * g + 16] = np.eye(16, dtype=np.float32)
    return xbs, wa, w2, id64, id128


def kernel(x, W):
    nc = _get_nc()
    xbs, wa, w2, id64, id128 = _host_layouts(x, W)
    in_maps = [
        {"xb": xbs[k], "wa": wa, "w2": w2, "id64": id64, "id128": id128}
        for k in range(NCORES)
    ]
    res = run_bass_kernel_spmd(nc, in_maps, core_ids=list(range(NCORES)))
    out = np.concatenate([r["v"] for r in res.results], axis=0)
    return np.ascontiguousarray(out.astype(np.float32))


if __name__ == "__main__":
    rng = np.random.default_rng(0)
    x = rng.standard_normal((B, 32, S, 6, 6), dtype=np.float32)
    W = rng.uniform(-1, 1, (1, O, I, OW, S)).astype(np.float32) / np.sqrt(S)
    v = kernel(x, W)
    print("out", v.shape, v.dtype, float(np.abs(v).max()))
